# revision 3
# baseline (speedup 1.0000x reference)
"""Trainium2 Bass kernel for a dense transformer block (B=8,S=1024,D=1024,H=16,FFN=4096)
with a parallel adapter. Data-parallel over batch: one batch element per NeuronCore.

Layout strategy: all activations live "transposed" on chip as [feature_partition, token_free]
([128, n_tiles, S] sbuf tiles). LayerNorm statistics are computed with PE ones-matmuls
(sum and sum-of-squares over the partition/feature axis), broadcast back over partitions
with rank-1 (K=1) matmuls, and applied with two DVE tensor-tensor ops. Attention scores
are computed directly in [key, query] layout so no transposes of the probability matrix
are ever needed; softmax denominators come from a ones-column appended to V; the
probabilities stay unnormalized until after P@V, where a rank-1 broadcast of the
reciprocal row-sum rescales the output. All per-channel biases are injected as rank-1
matmuls into the PSUM accumulation groups.
"""

import sys

sys.path.insert(0, "/opt/trn_rl_repo")

import numpy as np
import ml_dtypes

import concourse.bass as bass  # noqa: F401  (AP types)
import concourse.tile as tile
from concourse import bacc, mybir
from concourse.bass_utils import run_bass_kernel_spmd

BF = mybir.dt.bfloat16
F32 = mybir.dt.float32
BF_NP = ml_dtypes.bfloat16

B, S, D, H, HD, FFN, BN = 8, 1024, 1024, 16, 64, 4096, 64
KT = D // 128  # 8 feature tiles of the model dim
FT = FFN // 128  # 32 feature tiles of the ffn dim
NQ = S // 512  # 512-wide token slices
EPS = 1e-5
SCALE = HD**-0.5
ASCALE = 0.1


def _declare(nc, with_chain_input=False, suffix=""):
    t = {}

    def d(name, shape, dt, kind="ExternalInput"):
        t[name] = nc.dram_tensor(name + suffix, shape, dt, kind=kind).ap()

    d("xt", [128, KT, S], F32)
    d("wq", [128, KT, D], BF)
    d("wk", [128, KT, D], BF)
    d("wv", [128, KT, D], BF)
    d("wo", [128, KT, D], BF)
    d("w1", [128, KT, FFN], BF)
    d("w2", [KT, 128, FT, 128], BF)
    d("wd", [128, KT, BN], BF)
    d("wu", [BN, D], BF)
    d("bq", [D], F32)
    d("bk", [D], F32)
    d("b1x", [FFN], F32)
    d("bdx", [BN], F32)
    d("bvr", [1, D], BF)
    d("bor", [1, D], BF)
    d("fbr", [1, D], BF)
    d("out", [S, D], F32, kind="ExternalOutput")
    return t


def _emit(ctx, tc, t, perm, consts, xt_src=None, out_extra=None):
    """Emit one block's worth of instructions.

    t: dict of dram APs. perm: permanent pool. consts: dict with ones tiles + bias tiles.
    xt_src: optional override AP for the xt input (dram, [128, KT, S] f32).
    out_extra: optional dram AP [128, KT, S] f32 that also receives the packed output.
    """
    nc = tc.nc
    ones_row = consts["ones_row"]  # [1, 512] bf16 = 1.0
    ones_col = consts["ones_col"]  # [128, 1] bf16 = 1.0
    ones64 = consts["ones64"]  # [65, 64] bf16, row 64 = 1.0
    bq_sb = consts["bq_sb"]  # [128, KT] f32
    bk_sb = consts["bk_sb"]
    b1_sb = consts["b1_sb"]  # [128, FT] f32
    bd_sb = consts["bd_sb"]  # [BN, 1] f32
    bv_row = consts["bv_row"]  # [1, D] bf16
    bo_row = consts["bo_row"]
    fb_row = consts["fb_row"]

    x2T = perm.tile([128, KT, S], F32, tag="x2T")

    def layernorm(src, dst, name):
        """src: [128, KT, S] f32 AP; dst: [128, KT, S] bf16 tile. Plain LN core
        (gamma/beta are folded into the consumers by the host)."""
        with (
            tc.tile_pool(name=f"ln_{name}", bufs=3) as lp,
            tc.tile_pool(name=f"ln_{name}_r", bufs=1) as lr,
            tc.tile_pool(name=f"lnp_{name}", bufs=1, space="PSUM") as pp,
        ):
            s1 = pp.tile([1, S], F32, tag="s1")
            s2 = pp.tile([1, S], F32, tag="s2")
            for qq in range(NQ):
                ql = slice(qq * 512, (qq + 1) * 512)
                for kk in range(KT):
                    xb = lp.tile([128, 512], BF, tag="xb")
                    nc.scalar.copy(xb, src[:, kk, ql])
                    nc.tensor.matmul(
                        s1[0:1, ql], ones_col, xb,
                        start=kk == 0, stop=kk == KT - 1,
                    )
                for kk in range(KT):
                    xq = lp.tile([128, 512], BF, tag="xq")
                    nc.scalar.square(xq, src[:, kk, ql])
                    nc.tensor.matmul(
                        s2[0:1, ql], ones_col, xq,
                        start=kk == 0, stop=kk == KT - 1,
                    )
            m = lr.tile([1, S], F32, tag="m")
            ex2 = lr.tile([1, S], F32, tag="ex2")
            nc.vector.tensor_scalar_mul(m, s1[0:1, :], 1.0 / D)
            nc.vector.tensor_scalar_mul(ex2, s2[0:1, :], 1.0 / D)
            var = lr.tile([1, S], F32, tag="var")
            nc.vector.tensor_mul(var, m, m)
            nc.vector.tensor_sub(var, ex2, var)
            nc.vector.tensor_scalar_add(var, var, EPS)
            rv = lr.tile([1, S], F32, tag="rv")
            nc.vector.reciprocal(rv, var)
            rstd = lr.tile([1, S], F32, tag="rstd")
            nc.scalar.sqrt(rstd, rv)  # 1/sqrt(var+eps)
            nmrs = lr.tile([1, S], F32, tag="nmrs")
            nc.vector.tensor_mul(nmrs, m, rstd)
            rstd_bf = lr.tile([1, S], BF, tag="rstd_bf")
            nmrs_bf = lr.tile([1, S], BF, tag="nmrs_bf")
            nc.scalar.copy(rstd_bf, rstd)
            nc.scalar.activation(
                nmrs_bf, nmrs, mybir.ActivationFunctionType.Copy, scale=-1.0
            )
            rb = pp.tile([128, S], F32, tag="rb")
            mb = pp.tile([128, S], F32, tag="mb")
            for qq in range(NQ):
                ql = slice(qq * 512, (qq + 1) * 512)
                nc.tensor.matmul(
                    rb[:, ql], ones_row[0:1, 0:128], rstd_bf[0:1, ql],
                    start=True, stop=True,
                )
                nc.tensor.matmul(
                    mb[:, ql], ones_row[0:1, 0:128], nmrs_bf[0:1, ql],
                    start=True, stop=True,
                )
            for kk in range(KT):
                tmp = lp.tile([128, S], F32, tag="lntmp")
                nc.vector.tensor_mul(tmp, src[:, kk, :], rb)
                nc.vector.tensor_add(dst[:, kk, :], tmp, mb)

    # ================= attention =================
    with tc.tile_pool(name="attn_big", bufs=1) as ap_:
        xt_sb = ap_.tile([128, KT, S], F32, tag="xt")
        nc.sync.dma_start(xt_sb, xt_src if xt_src is not None else t["xt"])
        qT = ap_.tile([128, KT, S], BF, tag="qT")
        kT = ap_.tile([128, KT, S], BF, tag="kT")
        vS = ap_.tile([128, KT, H, HD + 1], BF, tag="vS")  # token-major V + ones col
        attnT = ap_.tile([128, KT, S], BF, tag="attnT")
        nc.vector.memset(vS[:, :, :, HD : HD + 1], 1.0)

        with tc.tile_pool(name="hT", bufs=1) as hp:
            hT = hp.tile([128, KT, S], BF, tag="hT")
            layernorm(xt_sb, hT, "ln1")
            with (
                tc.tile_pool(name="wqkv", bufs=1) as wp,
                tc.tile_pool(name="psqkv", bufs=4, space="PSUM") as qp,
            ):
                wq_sb = wp.tile([128, KT, D], BF, tag="wq")
                wk_sb = wp.tile([128, KT, D], BF, tag="wk")
                wv_sb = wp.tile([128, KT, D], BF, tag="wv")
                nc.sync.dma_start(wq_sb, t["wq"])
                nc.sync.dma_start(wk_sb, t["wk"])
                nc.sync.dma_start(wv_sb, t["wv"])
                for j in range(KT):
                    for ss in range(NQ):
                        sl = slice(ss * 512, (ss + 1) * 512)
                        jl = slice(j * 128, (j + 1) * 128)
                        psq = qp.tile([128, 512], F32, tag="ps")
                        for kk in range(KT):
                            nc.tensor.matmul(
                                psq, wq_sb[:, kk, jl], hT[:, kk, sl],
                                start=kk == 0, stop=kk == KT - 1,
                            )
                        nc.scalar.activation(
                            qT[:, j, sl], psq,
                            mybir.ActivationFunctionType.Identity,
                            bias=bq_sb[:, j : j + 1],
                        )
                        psk = qp.tile([128, 512], F32, tag="ps")
                        for kk in range(KT):
                            nc.tensor.matmul(
                                psk, wk_sb[:, kk, jl], hT[:, kk, sl],
                                start=kk == 0, stop=kk == KT - 1,
                            )
                        nc.scalar.activation(
                            kT[:, j, sl], psk,
                            mybir.ActivationFunctionType.Identity,
                            bias=bk_sb[:, j : j + 1],
                        )
                for si in range(KT):
                    il = slice(si * 128, (si + 1) * 128)
                    for dd in range(NQ):
                        dl = slice(dd * 512, (dd + 1) * 512)
                        psv = qp.tile([128, 512], F32, tag="ps")
                        for kk in range(KT):
                            nc.tensor.matmul(
                                psv, hT[:, kk, il], wv_sb[:, kk, dl],
                                start=kk == 0, stop=False,
                            )
                        nc.tensor.matmul(
                            psv, ones_row[0:1, 0:128], bv_row[0:1, dl],
                            start=False, stop=True,
                        )
                        nc.scalar.copy(
                            vS[:, si, dd * 8 : (dd + 1) * 8, 0:HD],
                            psv.rearrange("p (h e) -> p h e", h=8),
                        )

        # scores -> exp -> P@V -> normalize
        with (
            tc.tile_pool(name="pt", bufs=2) as ptp,
            tc.tile_pool(name="att_sm", bufs=2) as smp,
            tc.tile_pool(name="ps_sc", bufs=4, space="PSUM") as scp,
            tc.tile_pool(name="ps_pv", bufs=2, space="PSUM") as pvp,
            tc.tile_pool(name="ps_bc", bufs=2, space="PSUM") as bcp,
        ):
            for tp in range(KT):  # head pair
                for qq in range(NQ):
                    ql = slice(qq * 512, (qq + 1) * 512)
                    ptA = ptp.tile([128, KT, 512], BF, tag="ptA")
                    ptB = ptp.tile([128, KT, 512], BF, tag="ptB")
                    for kk in range(KT):
                        kl = slice(kk * 128, (kk + 1) * 128)
                        psA = scp.tile([128, 512], F32, tag="sc")
                        psB = scp.tile([128, 512], F32, tag="sc")
                        nc.tensor.matmul(
                            psA, kT[0:64, tp, kl], qT[0:64, tp, ql],
                            start=True, stop=True,
                        )
                        nc.tensor.matmul(
                            psB, kT[64:128, tp, kl], qT[64:128, tp, ql],
                            start=True, stop=True,
                        )
                        nc.scalar.activation(
                            ptA[:, kk, :], psA,
                            mybir.ActivationFunctionType.Exp, scale=SCALE,
                        )
                        nc.scalar.activation(
                            ptB[:, kk, :], psB,
                            mybir.ActivationFunctionType.Exp, scale=SCALE,
                        )
                    pvA = pvp.tile([65, 512], F32, tag="pv")
                    pvB = pvp.tile([65, 512], F32, tag="pv")
                    for kk in range(KT):
                        nc.tensor.matmul(
                            pvA, vS[:, kk, 2 * tp, :], ptA[:, kk, :],
                            start=kk == 0, stop=kk == KT - 1,
                        )
                        nc.tensor.matmul(
                            pvB, vS[:, kk, 2 * tp + 1, :], ptB[:, kk, :],
                            start=kk == 0, stop=kk == KT - 1,
                        )
                    for h01, pv in ((0, pvA), (1, pvB)):
                        au = smp.tile([64, 512], BF, tag="au")
                        nc.scalar.copy(au, pv[0:64, :])
                        rr = smp.tile([65, 512], F32, tag="rr")
                        nc.vector.reciprocal(rr[64:65, :], pv[64:65, :])
                        rrb = smp.tile([65, 512], BF, tag="rrb")
                        nc.scalar.copy(rrb[64:65, :], rr[64:65, :])
                        bc = bcp.tile([64, 512], F32, tag="bc")
                        nc.tensor.matmul(
                            bc, ones64[64:65, 0:64], rrb[64:65, :],
                            start=True, stop=True,
                        )
                        if h01 == 0:
                            nc.vector.tensor_mul(attnT[0:64, tp, ql], au, bc)
                        else:
                            tmp2 = smp.tile([64, 512], BF, tag="tmp2")
                            nc.vector.tensor_mul(tmp2, au, bc)
                            nc.sync.dma_start(attnT[64:128, tp, ql], tmp2)

        # out-projection + residual -> x2T (f32)
        with (
            tc.tile_pool(name="wo", bufs=1) as wop,
            tc.tile_pool(name="ps_wo", bufs=4, space="PSUM") as wpp,
        ):
            wo_sb = wop.tile([128, KT, D], BF, tag="wo")
            nc.sync.dma_start(wo_sb, t["wo"])
            for j in range(KT):
                jl = slice(j * 128, (j + 1) * 128)
                for qq in range(NQ):
                    ql = slice(qq * 512, (qq + 1) * 512)
                    ps = wpp.tile([128, 512], F32, tag="ps")
                    for kk in range(KT):
                        nc.tensor.matmul(
                            ps, wo_sb[:, kk, jl], attnT[:, kk, ql],
                            start=kk == 0, stop=False,
                        )
                    nc.tensor.matmul(
                        ps, bo_row[0:1, jl], ones_row[0:1, 0:512],
                        start=False, stop=True,
                    )
                    nc.vector.tensor_add(
                        x2T[:, j, ql], xt_sb[:, j, ql], ps
                    )

    # ================= MLP + adapter =================
    with tc.tile_pool(name="ffn_big", bufs=1) as fbp:
        zT = fbp.tile([128, FT, S], BF, tag="zT")
        rT = fbp.tile([BN, S], BF, tag="rT")
        with tc.tile_pool(name="nT", bufs=1) as ntp:
            nT = ntp.tile([128, KT, S], BF, tag="nT")
            layernorm(x2T, nT, "ln2")
            with (
                tc.tile_pool(name="w1s", bufs=2) as w1p,
                tc.tile_pool(name="ps_u", bufs=4, space="PSUM") as pup,
            ):
                for fq in range(4):  # stream w1 in quarters
                    w1_q = w1p.tile([128, KT, 1024], BF, tag="w1q")
                    nc.sync.dma_start(
                        w1_q, t["w1"][:, :, fq * 1024 : (fq + 1) * 1024]
                    )
                    for fl in range(8):
                        f = fq * 8 + fl
                        fsl = slice(fl * 128, (fl + 1) * 128)
                        for ss in range(NQ):
                            sl = slice(ss * 512, (ss + 1) * 512)
                            psu = pup.tile([128, 512], F32, tag="pu")
                            for kk in range(KT):
                                nc.tensor.matmul(
                                    psu, w1_q[:, kk, fsl], nT[:, kk, sl],
                                    start=kk == 0, stop=kk == KT - 1,
                                )
                            nc.scalar.activation(
                                zT[:, f, sl], psu,
                                mybir.ActivationFunctionType.Gelu,
                                bias=b1_sb[:, f : f + 1],
                            )
                # adapter down + relu
                with tc.tile_pool(name="wds", bufs=1) as wdp:
                    wd_sb = wdp.tile([128, KT, BN], BF, tag="wd")
                    nc.sync.dma_start(wd_sb, t["wd"])
                    for qq in range(NQ):
                        ql = slice(qq * 512, (qq + 1) * 512)
                        psd = pup.tile([BN, 512], F32, tag="pd")
                        for kk in range(KT):
                            nc.tensor.matmul(
                                psd, wd_sb[:, kk, :], nT[:, kk, ql],
                                start=kk == 0, stop=kk == KT - 1,
                            )
                        nc.scalar.activation(
                            rT[0:BN, ql], psd,
                            mybir.ActivationFunctionType.Relu,
                            bias=bd_sb[:, 0:1],
                        )
        with (
            tc.tile_pool(name="w2s", bufs=2) as w2p,
            tc.tile_pool(name="wus", bufs=1) as wup,
            tc.tile_pool(name="outs", bufs=3) as otp,
            tc.tile_pool(name="ps_y", bufs=4, space="PSUM") as pyp,
        ):
            wu_sb = wup.tile([BN, D], BF, tag="wu")
            nc.sync.dma_start(wu_sb, t["wu"])
            for j in range(KT):
                jl = slice(j * 128, (j + 1) * 128)
                w2_j = w2p.tile([128, FT, 128], BF, tag="w2j")
                nc.sync.dma_start(w2_j, t["w2"][j])
                for qq in range(NQ):
                    ql = slice(qq * 512, (qq + 1) * 512)
                    psy = pyp.tile([128, 512], F32, tag="py")
                    for f in range(FT):
                        nc.tensor.matmul(
                            psy, w2_j[:, f, :], zT[:, f, ql],
                            start=f == 0, stop=False,
                        )
                    nc.tensor.matmul(
                        psy, wu_sb[0:BN, jl], rT[0:BN, ql],
                        start=False, stop=False,
                    )
                    nc.tensor.matmul(
                        psy, fb_row[0:1, jl], ones_row[0:1, 0:512],
                        start=False, stop=True,
                    )
                    ot = otp.tile([128, 512], F32, tag="ot")
                    nc.vector.tensor_add(ot, x2T[:, j, ql], psy)
                    nc.sync.dma_start(
                        t["out"].rearrange("s (j p) -> p j s", p=128)[:, j, ql], ot
                    )
                    if out_extra is not None:
                        nc.sync.dma_start(out_extra[:, j, ql], ot)


def _build(dup=1):
    nc = bacc.Bacc("TRN2", target_bir_lowering=False, debug=False, num_devices=8)
    t = _declare(nc)
    chain = []
    for i in range(max(0, dup - 1)):
        chain.append(
            nc.dram_tensor(f"xchain{i}", [128, KT, S], F32, kind="Internal").ap()
        )
    with tile.TileContext(nc) as tc:
        from contextlib import ExitStack

        with ExitStack() as ctx:
            perm = ctx.enter_context(tc.tile_pool(name="perm", bufs=1))
            consts = {}
            ones_row = perm.tile([1, 512], BF, tag="ones_row")
            nc.vector.memset(ones_row, 1.0)
            ones_col = perm.tile([128, 1], BF, tag="ones_col")
            nc.vector.memset(ones_col, 1.0)
            ones64 = perm.tile([65, 64], BF, tag="ones64")
            nc.vector.memset(ones64[64:65, :], 1.0)
            consts.update(ones_row=ones_row, ones_col=ones_col, ones64=ones64)
            bq_sb = perm.tile([128, KT], F32, tag="bq")
            nc.sync.dma_start(bq_sb, t["bq"].rearrange("(j p) -> p j", p=128))
            bk_sb = perm.tile([128, KT], F32, tag="bk")
            nc.sync.dma_start(bk_sb, t["bk"].rearrange("(j p) -> p j", p=128))
            b1_sb = perm.tile([128, FT], F32, tag="b1")
            nc.sync.dma_start(b1_sb, t["b1x"].rearrange("(j p) -> p j", p=128))
            bd_sb = perm.tile([BN, 1], F32, tag="bd")
            nc.sync.dma_start(bd_sb, t["bdx"].rearrange("(p o) -> p o", o=1))
            bv_row = perm.tile([1, D], BF, tag="bv")
            nc.sync.dma_start(bv_row, t["bvr"])
            bo_row = perm.tile([1, D], BF, tag="bo")
            nc.sync.dma_start(bo_row, t["bor"])
            fb_row = perm.tile([1, D], BF, tag="fb")
            nc.sync.dma_start(fb_row, t["fbr"])
            consts.update(
                bq_sb=bq_sb, bk_sb=bk_sb, b1_sb=b1_sb, bd_sb=bd_sb,
                bv_row=bv_row, bo_row=bo_row, fb_row=fb_row,
            )
            for i in range(dup):
                src = None if i == 0 else chain[i - 1]
                extra = chain[i] if i < dup - 1 else None
                _emit(ctx, tc, t, perm, consts, xt_src=src, out_extra=extra)
    nc.compile()
    return nc


_nc_cache = {}


def _get_nc(dup=1):
    if dup not in _nc_cache:
        _nc_cache[dup] = _build(dup)
    return _nc_cache[dup]


def _pack_feat(w):
    """[D_in, O] -> [128, D_in//128, O]"""
    din, o = w.shape
    return np.ascontiguousarray(w.reshape(din // 128, 128, o).transpose(1, 0, 2))


def prepare_inputs(inputs):
    """Host-side folding/packing. Returns (shared dict, per-core xt list)."""
    f32 = np.float32
    x = np.asarray(inputs["x"], f32)
    ln1_g, ln1_b = np.asarray(inputs["ln1_g"], f32), np.asarray(inputs["ln1_b"], f32)
    ln2_g, ln2_b = np.asarray(inputs["ln2_g"], f32), np.asarray(inputs["ln2_b"], f32)
    aln_g, aln_b = np.asarray(inputs["aln_g"], f32), np.asarray(inputs["aln_b"], f32)
    wq, wk, wv, wo = (np.asarray(inputs[k], f32) for k in ("wq", "wk", "wv", "wo"))
    w1, w2 = np.asarray(inputs["w1"], f32), np.asarray(inputs["w2"], f32)
    wd, wu = np.asarray(inputs["wd"], f32), np.asarray(inputs["wu"], f32)
    b1, b2 = np.asarray(inputs["b1"], f32), np.asarray(inputs["b2"], f32)
    bd, bu = np.asarray(inputs["bd"], f32), np.asarray(inputs["bu"], f32)
    bo = np.asarray(inputs["bo"], f32)

    shared = {
        "wq": _pack_feat(ln1_g[:, None] * wq).astype(BF_NP),
        "wk": _pack_feat(ln1_g[:, None] * wk).astype(BF_NP),
        "wv": _pack_feat(ln1_g[:, None] * wv).astype(BF_NP),
        "wo": _pack_feat(wo).astype(BF_NP),
        "w1": _pack_feat(ln2_g[:, None] * w1).astype(BF_NP),
        "w2": np.ascontiguousarray(
            w2.reshape(FT, 128, KT, 128).transpose(2, 1, 0, 3)
        ).astype(BF_NP),
        "wd": _pack_feat(aln_g[:, None] * wd).astype(BF_NP),
        "wu": (ASCALE * wu).astype(BF_NP),
        "bq": ln1_b @ wq,
        "bk": ln1_b @ wk,
        "b1x": b1 + ln2_b @ w1,
        "bdx": bd + aln_b @ wd,
        "bvr": (ln1_b @ wv)[None, :].astype(BF_NP),
        "bor": bo[None, :].astype(BF_NP),
        "fbr": (b2 + ASCALE * bu)[None, :].astype(BF_NP),
    }
    xts = [
        np.ascontiguousarray(
            x[c].T.reshape(KT, 128, S).transpose(1, 0, 2)
        )  # [128, KT, S] where [p, kk, s] = x[c, s, kk*128+p]
        for c in range(B)
    ]
    return shared, xts


def kernel(**inputs):
    nc = _get_nc(dup=1)
    shared, xts = prepare_inputs(inputs)
    in_maps = [{**shared, "xt": xts[c]} for c in range(B)]
    res = run_bass_kernel_spmd(nc, in_maps, core_ids=list(range(B)))
    out = np.stack([res.results[c]["out"] for c in range(B)], axis=0)
    return out.astype(np.float32)


# revision 38
# speedup vs baseline: 3.9251x; 3.9251x over previous
"""Trainium2 Bass kernel for a dense transformer block (B=8,S=1024,D=1024,H=16,FFN=4096)
with a parallel adapter. Data-parallel over batch: one batch element per NeuronCore.

Layout strategy: all activations live "transposed" on chip as [feature_partition, token_free]
([128, n_tiles, S] sbuf tiles). LayerNorm statistics are computed with PE ones-matmuls
(sum and sum-of-squares over the partition/feature axis), broadcast back over partitions
with rank-1 (K=1) matmuls, and applied with two DVE tensor-tensor ops. Attention scores
are computed directly in [key, query] layout so no transposes of the probability matrix
are ever needed; softmax denominators come from a ones-column appended to V; the
probabilities stay unnormalized until after P@V, where a rank-1 broadcast of the
reciprocal row-sum rescales the output. All per-channel biases are injected as rank-1
matmuls into the PSUM accumulation groups.
"""

import sys

sys.path.insert(0, "/opt/trn_rl_repo")

import numpy as np
import ml_dtypes

import concourse.bass as bass  # noqa: F401  (AP types)
import concourse.tile as tile
from concourse import bacc, mybir
from concourse.bass_utils import run_bass_kernel_spmd

BF = mybir.dt.bfloat16
F32 = mybir.dt.float32
BF_NP = ml_dtypes.bfloat16

B, S, D, H, HD, FFN, BN = 8, 1024, 1024, 16, 64, 4096, 64
KT = D // 128  # 8 feature tiles of the model dim
FT = FFN // 128  # 32 feature tiles of the ffn dim
NQ = S // 512  # 512-wide token slices
EPS = 1e-5
SCALE = HD**-0.5
ASCALE = 0.1


def _declare(nc, with_chain_input=False, suffix=""):
    t = {}

    def d(name, shape, dt, kind="ExternalInput"):
        t[name] = nc.dram_tensor(name + suffix, shape, dt, kind=kind).ap()

    d("xt", [128, KT, S], F32)
    d("wq", [128, KT, D], BF)
    d("wk", [128, KT, D], BF)
    d("wv", [128, KT, D], BF)
    d("wo", [128, KT, D], BF)
    d("w1", [128, KT, FFN], BF)
    d("w2", [KT, 128, FT, 128], BF)
    d("wd", [128, KT, BN], BF)
    d("wu", [BN, D], BF)
    d("bq", [D], F32)
    d("bk", [D], F32)
    d("b1x", [FFN], F32)
    d("bdx", [BN], F32)
    d("bvr", [1, D], BF)
    d("bor", [1, D], BF)
    d("fbr", [1, D], BF)
    # packed transposed output [p, kk, s] = out[s, kk*128+p]; host untransposes.
    d("out", [128, KT, S], F32, kind="ExternalOutput")
    return t


def _emit(ctx, tc, t, perm, consts, xt_src=None, out_extra=None, mode=5):
    """Emit one block's worth of instructions.

    t: dict of dram APs. perm: permanent pool. consts: dict with ones tiles + bias tiles.
    xt_src: optional override AP for the xt input (dram, [128, KT, S] f32).
    out_extra: optional dram AP [128, KT, S] f32 that also receives the packed output.
    """
    nc = tc.nc
    ones_row = consts["ones_row"]  # [1, 512] bf16 = 1.0
    ones_col = consts["ones_col"]  # [128, 1] bf16 = 1.0
    ones64 = consts["ones64"]  # [65, 64] bf16, row 64 = 1.0
    bq_sb = consts["bq_sb"]  # [128, KT] f32
    bk_sb = consts["bk_sb"]
    b1_sb = consts["b1_sb"]  # [128, FT] f32
    bd_sb = consts["bd_sb"]  # [BN, 1] f32
    bv_row = consts["bv_row"]  # [1, D] bf16
    bo_row = consts["bo_row"]
    fb_row = consts["fb_row"]

    x2T = perm.tile([128, KT, S], F32, tag="x2T")

    def dump_bf(tile3d, nt):
        """Debug/bisection: stage a [128, nt, S] tile to out (packed layout)."""
        with tc.tile_pool(name="dump", bufs=3) as dp:
            for j in range(min(nt, KT)):
                for qq in range(NQ):
                    ql = slice(qq * 512, (qq + 1) * 512)
                    st = dp.tile([128, 512], F32, tag="st")
                    nc.scalar.copy(st, tile3d[:, j, ql])
                    nc.sync.dma_start(t["out"][:, j, ql], st)
                    if out_extra is not None:
                        nc.sync.dma_start(out_extra[:, j, ql], st)

    def layernorm(src, dst, name):
        """src: [128, KT, S] f32 AP; dst: [128, KT, S] bf16 tile. Plain LN core
        (gamma/beta are folded into the consumers by the host)."""
        with (
            tc.tile_pool(name=f"ln_{name}", bufs=3) as lp,
            tc.tile_pool(name=f"ln_{name}_r", bufs=1) as lr,
            tc.tile_pool(name=f"lnp_{name}", bufs=1, space="PSUM") as pp,
        ):
            s1 = pp.tile([1, S], F32, tag="s1")
            s2 = pp.tile([1, S], F32, tag="s2")
            for qq in range(NQ):
                ql = slice(qq * 512, (qq + 1) * 512)
                for kk in range(KT):
                    xb = lp.tile([128, 512], BF, tag="xb")
                    nc.scalar.copy(xb, src[:, kk, ql])
                    nc.tensor.matmul(
                        s1[0:1, ql], ones_col, xb,
                        start=kk == 0, stop=kk == KT - 1,
                    )
                for kk in range(KT):
                    xq = lp.tile([128, 512], BF, tag="xq")
                    nc.scalar.square(xq, src[:, kk, ql])
                    nc.tensor.matmul(
                        s2[0:1, ql], ones_col, xq,
                        start=kk == 0, stop=kk == KT - 1,
                    )
            m = lr.tile([1, S], F32, tag="m")
            ex2 = lr.tile([1, S], F32, tag="ex2")
            nc.vector.tensor_scalar_mul(m, s1[0:1, :], 1.0 / D)
            nc.vector.tensor_scalar_mul(ex2, s2[0:1, :], 1.0 / D)
            var = lr.tile([1, S], F32, tag="var")
            nc.vector.tensor_mul(var, m, m)
            nc.vector.tensor_sub(var, ex2, var)
            nc.vector.tensor_scalar_add(var, var, EPS)
            rv = lr.tile([1, S], F32, tag="rv")
            nc.vector.reciprocal(rv, var)
            rstd = lr.tile([1, S], F32, tag="rstd")
            nc.scalar.sqrt(rstd, rv)  # 1/sqrt(var+eps)
            nmrs = lr.tile([1, S], F32, tag="nmrs")
            nc.vector.tensor_mul(nmrs, m, rstd)
            rstd_bf = lr.tile([1, S], BF, tag="rstd_bf")
            nmrs_bf = lr.tile([1, S], BF, tag="nmrs_bf")
            nc.scalar.copy(rstd_bf, rstd)
            nc.scalar.activation(
                nmrs_bf, nmrs, mybir.ActivationFunctionType.Copy, scale=-1.0
            )
            rb = pp.tile([128, S], F32, tag="rb")
            mb = pp.tile([128, S], F32, tag="mb")
            for qq in range(NQ):
                ql = slice(qq * 512, (qq + 1) * 512)
                nc.tensor.matmul(
                    rb[:, ql], ones_row[0:1, 0:128], rstd_bf[0:1, ql],
                    start=True, stop=True,
                )
                nc.tensor.matmul(
                    mb[:, ql], ones_row[0:1, 0:128], nmrs_bf[0:1, ql],
                    start=True, stop=True,
                )
            for kk in range(KT):
                tmp = lp.tile([128, S], F32, tag="lntmp")
                nc.vector.tensor_mul(tmp, src[:, kk, :], rb)
                nc.vector.tensor_add(dst[:, kk, :], tmp, mb)

    # ================= attention =================
    with tc.tile_pool(name="attn_big", bufs=1) as ap_:
        xt_sb = ap_.tile([128, KT, S], F32, tag="xt")
        xsrc = xt_src if xt_src is not None else t["xt"]
        for kk in range(KT):
            eng = nc.sync if kk % 2 == 0 else nc.gpsimd
            eng.dma_start(xt_sb[:, kk, :], xsrc[:, kk, :])
        if mode == 0:
            dump_bf(xt_sb, KT)
            return
        qT = ap_.tile([128, KT, S], BF, tag="qT")
        kT = ap_.tile([128, KT, S], BF, tag="kT")
        vS = ap_.tile([128, KT, H, HD + 1], BF, tag="vS")  # token-major V + ones col
        attnT = ap_.tile([128, KT, S], BF, tag="attnT")
        nc.vector.memset(vS[:, :, :, HD : HD + 1], 1.0)

        with tc.tile_pool(name="hT", bufs=1) as hp:
            hT = hp.tile([128, KT, S], BF, tag="hT")
            layernorm(xt_sb, hT, "ln1")
            with (
                tc.tile_pool(name="wqkv", bufs=1) as wp,
                tc.tile_pool(name="psqkv", bufs=4, space="PSUM") as qp,
            ):
                wq_sb = wp.tile([128, KT, D], BF, tag="wq")
                wk_sb = wp.tile([128, KT, D], BF, tag="wk")
                wv_sb = wp.tile([128, KT, D], BF, tag="wv")
                for kk in range(KT):  # chunked across both HWDGE engines
                    eng = nc.sync if kk % 2 == 0 else nc.gpsimd
                    eng.dma_start(wq_sb[:, kk, :], t["wq"][:, kk, :])
                    eng.dma_start(wk_sb[:, kk, :], t["wk"][:, kk, :])
                    eng.dma_start(wv_sb[:, kk, :], t["wv"][:, kk, :])
                for j in range(KT):
                    jl = slice(j * 128, (j + 1) * 128)
                    psq = qp.tile([128, S], F32, tag="ps")
                    psk = qp.tile([128, S], F32, tag="ps")
                    for kk in range(KT):
                        for ss in range(NQ):  # weight reuse across slices
                            sl = slice(ss * 512, (ss + 1) * 512)
                            nc.tensor.matmul(
                                psq[:, sl], wq_sb[:, kk, jl], hT[:, kk, sl],
                                start=kk == 0, stop=kk == KT - 1,
                            )
                        for ss in range(NQ):
                            sl = slice(ss * 512, (ss + 1) * 512)
                            nc.tensor.matmul(
                                psk[:, sl], wk_sb[:, kk, jl], hT[:, kk, sl],
                                start=kk == 0, stop=kk == KT - 1,
                            )
                    nc.vector.tensor_scalar_add(qT[:, j, :], psq, bq_sb[:, j : j + 1])
                    nc.vector.tensor_scalar_add(kT[:, j, :], psk, bk_sb[:, j : j + 1])
                for si in range(KT):
                    il = slice(si * 128, (si + 1) * 128)
                    psv = qp.tile([128, S], F32, tag="ps")
                    for kk in range(KT):
                        for dd in range(NQ):  # hT stationary reused
                            dl = slice(dd * 512, (dd + 1) * 512)
                            nc.tensor.matmul(
                                psv[:, dl], hT[:, kk, il], wv_sb[:, kk, dl],
                                start=kk == 0, stop=False,
                            )
                    for dd in range(NQ):
                        dl = slice(dd * 512, (dd + 1) * 512)
                        nc.tensor.matmul(
                            psv[:, dl], ones_row[0:1, 0:128], bv_row[0:1, dl],
                            start=False, stop=True,
                        )
                    nc.vector.tensor_copy(
                        vS[:, si, :, 0:HD],
                        psv.rearrange("p (h e) -> p h e", h=H),
                    )

        if mode == 1:
            dump_bf(qT, KT)
            return

        # scores -> exp -> P@V -> normalize
        with (
            tc.tile_pool(name="pt", bufs=1) as ptp,
            tc.tile_pool(name="att_sm", bufs=2) as smp,
            tc.tile_pool(name="rs_dram", bufs=2, space="DRAM") as rdp,
            tc.tile_pool(name="ps_sc", bufs=2, space="PSUM") as scp,
            tc.tile_pool(name="ps_pv", bufs=2, space="PSUM") as pvp,
        ):
            for tp in range(KT):  # head pair
                ptA = ptp.tile([128, KT, S], BF, tag="ptA")
                ptB = ptp.tile([128, KT, S], BF, tag="ptB")
                for kk in range(KT):
                    kl = slice(kk * 128, (kk + 1) * 128)
                    psA = scp.tile([128, S], F32, tag="sc")
                    psB = scp.tile([128, S], F32, tag="sc")
                    for qq in range(NQ):  # kT stationary reused; A/B row-packed
                        ql = slice(qq * 512, (qq + 1) * 512)
                        nc.tensor.matmul(
                            psA[:, ql], kT[0:64, tp, kl], qT[0:64, tp, ql],
                            start=True, stop=True,
                        )
                    for qq in range(NQ):
                        ql = slice(qq * 512, (qq + 1) * 512)
                        nc.tensor.matmul(
                            psB[:, ql], kT[64:128, tp, kl], qT[64:128, tp, ql],
                            start=True, stop=True,
                        )
                    nc.scalar.activation(
                        ptA[:, kk, :], psA,
                        mybir.ActivationFunctionType.Exp, scale=SCALE,
                    )
                    nc.scalar.activation(
                        ptB[:, kk, :], psB,
                        mybir.ActivationFunctionType.Exp, scale=SCALE,
                    )
                pvA = pvp.tile([65, S], F32, tag="pv")
                pvB = pvp.tile([65, S], F32, tag="pv")
                for kk in range(KT):
                    for qq in range(NQ):  # V stationary reused
                        ql = slice(qq * 512, (qq + 1) * 512)
                        nc.tensor.matmul(
                            pvA[:, ql], vS[:, kk, 2 * tp, :], ptA[:, kk, ql],
                            start=kk == 0, stop=kk == KT - 1,
                        )
                    for qq in range(NQ):
                        ql = slice(qq * 512, (qq + 1) * 512)
                        nc.tensor.matmul(
                            pvB[:, ql], vS[:, kk, 2 * tp + 1, :], ptB[:, kk, ql],
                            start=kk == 0, stop=kk == KT - 1,
                        )
                for h01, pv in ((0, pvA), (1, pvB)):
                    au = smp.tile([64, S], BF, tag="au")
                    nc.vector.tensor_copy(au, pv[0:64, :])
                    rr = smp.tile([65, S], F32, tag="rr")
                    nc.vector.reciprocal(rr[64:65, :], pv[64:65, :])
                    rb = smp.tile([64, S], F32, tag="rbc")
                    # broadcast the reciprocal row over 64 partitions via a
                    # DRAM round-trip (DRAM APs support step-0 partition dims)
                    rs_d = rdp.tile([1, S], F32, tag="rsd")
                    nc.gpsimd.dma_start(rs_d, rr[64:65, :])
                    rs_b = bass.AP(
                        tensor=rs_d.tensor, offset=rs_d.offset,
                        ap=[[0, 64]] + list(rs_d.ap[1:]),
                    )
                    nc.sync.dma_start(rb, rs_b)
                    if h01 == 0:
                        nc.vector.tensor_mul(attnT[0:64, tp, :], au, rb)
                    else:
                        tmp2 = smp.tile([64, S], BF, tag="tmp2")
                        nc.vector.tensor_mul(tmp2, au, rb)
                        nc.sync.dma_start(attnT[64:128, tp, :], tmp2)

        if mode == 2:
            dump_bf(attnT, KT)
            return

        # out-projection + residual -> x2T (f32)
        with (
            tc.tile_pool(name="wo", bufs=1) as wop,
            tc.tile_pool(name="ps_wo", bufs=4, space="PSUM") as wpp,
        ):
            wo_sb = wop.tile([128, KT, D], BF, tag="wo")
            for kk in range(KT):
                eng = nc.sync if kk % 2 == 0 else nc.gpsimd
                eng.dma_start(wo_sb[:, kk, :], t["wo"][:, kk, :])
            for j in range(KT):
                jl = slice(j * 128, (j + 1) * 128)
                ps = wpp.tile([128, S], F32, tag="ps")
                for kk in range(KT):
                    for qq in range(NQ):
                        ql = slice(qq * 512, (qq + 1) * 512)
                        nc.tensor.matmul(
                            ps[:, ql], wo_sb[:, kk, jl], attnT[:, kk, ql],
                            start=kk == 0, stop=False,
                        )
                for qq in range(NQ):
                    ql = slice(qq * 512, (qq + 1) * 512)
                    nc.tensor.matmul(
                        ps[:, ql], bo_row[0:1, jl], ones_row[0:1, 0:512],
                        start=False, stop=True,
                    )
                nc.vector.tensor_add(x2T[:, j, :], xt_sb[:, j, :], ps)

    if mode == 3:
        dump_bf(x2T, KT)
        return

    # ================= MLP + adapter =================
    with tc.tile_pool(name="ffn_big", bufs=1) as fbp:
        zT = fbp.tile([128, FT, S], BF, tag="zT")
        rT = fbp.tile([BN, S], BF, tag="rT")
        with tc.tile_pool(name="nT", bufs=1) as ntp:
            nT = ntp.tile([128, KT, S], BF, tag="nT")
            layernorm(x2T, nT, "ln2")
            with (
                tc.tile_pool(name="w1s", bufs=2) as w1p,
                tc.tile_pool(name="ps_u", bufs=3, space="PSUM") as pup,
            ):
                for fq in range(4):  # stream w1 in quarters
                    w1_q = w1p.tile([128, KT, 1024], BF, tag="w1q")
                    for kk in range(KT):
                        eng = nc.sync if kk % 2 == 0 else nc.gpsimd
                        eng.dma_start(
                            w1_q[:, kk, :],
                            t["w1"][:, kk, fq * 1024 : (fq + 1) * 1024],
                        )
                    for fl in range(8):
                        f = fq * 8 + fl
                        fsl = slice(fl * 128, (fl + 1) * 128)
                        psu = pup.tile([128, S], F32, tag="pu")
                        for kk in range(KT):
                            for ss in range(NQ):  # w1 stationary reused
                                sl = slice(ss * 512, (ss + 1) * 512)
                                nc.tensor.matmul(
                                    psu[:, sl], w1_q[:, kk, fsl], nT[:, kk, sl],
                                    start=kk == 0, stop=kk == KT - 1,
                                )
                        nc.scalar.activation(
                            zT[:, f, :], psu,
                            mybir.ActivationFunctionType.Gelu,
                            bias=b1_sb[:, f : f + 1],
                        )
                # adapter down + relu
                with tc.tile_pool(name="wds", bufs=1) as wdp:
                    wd_sb = wdp.tile([128, KT, BN], BF, tag="wd")
                    nc.sync.dma_start(wd_sb, t["wd"])
                    psd = pup.tile([BN, S], F32, tag="pd", bufs=1)
                    for kk in range(KT):
                        for qq in range(NQ):
                            ql = slice(qq * 512, (qq + 1) * 512)
                            nc.tensor.matmul(
                                psd[:, ql], wd_sb[:, kk, :], nT[:, kk, ql],
                                start=kk == 0, stop=kk == KT - 1,
                            )
                    nc.scalar.activation(
                        rT[0:BN, :], psd,
                        mybir.ActivationFunctionType.Relu,
                        bias=bd_sb[:, 0:1],
                    )
        if mode == 4:
            dump_bf(zT, KT)
            return

        with (
            tc.tile_pool(name="w2s", bufs=2) as w2p,
            tc.tile_pool(name="wus", bufs=1) as wup,
            tc.tile_pool(name="outs", bufs=3) as otp,
            tc.tile_pool(name="ps_y", bufs=4, space="PSUM") as pyp,
        ):
            wu_sb = wup.tile([BN, D], BF, tag="wu")
            nc.sync.dma_start(wu_sb, t["wu"])
            for j in range(KT):
                jl = slice(j * 128, (j + 1) * 128)
                w2_j = w2p.tile([128, FT, 128], BF, tag="w2j")
                for fh in range(4):
                    eng = nc.sync if fh % 2 == 0 else nc.gpsimd
                    fsl = slice(fh * 8, (fh + 1) * 8)
                    eng.dma_start(w2_j[:, fsl, :], t["w2"][j][:, fsl, :])
                psy = pyp.tile([128, S], F32, tag="py")
                for f in range(FT):
                    for qq in range(NQ):  # w2 stationary reused
                        ql = slice(qq * 512, (qq + 1) * 512)
                        nc.tensor.matmul(
                            psy[:, ql], w2_j[:, f, :], zT[:, f, ql],
                            start=f == 0, stop=False,
                        )
                for qq in range(NQ):
                    ql = slice(qq * 512, (qq + 1) * 512)
                    nc.tensor.matmul(
                        psy[:, ql], wu_sb[0:BN, jl], rT[0:BN, ql],
                        start=False, stop=False,
                    )
                    nc.tensor.matmul(
                        psy[:, ql], fb_row[0:1, jl], ones_row[0:1, 0:512],
                        start=False, stop=True,
                    )
                ot = otp.tile([128, S], F32, tag="ot")
                nc.vector.tensor_add(ot, x2T[:, j, :], psy)
                eng = nc.sync if j % 2 == 0 else nc.gpsimd
                eng.dma_start(t["out"][:, j, :], ot)
                if out_extra is not None:
                    eng.dma_start(out_extra[:, j, :], ot)


def _build(dup=1, mode=5):
    nc = bacc.Bacc("TRN2", target_bir_lowering=False, debug=False, num_devices=8)
    t = _declare(nc)
    chain = []
    for i in range(max(0, dup - 1)):
        chain.append(
            nc.dram_tensor(f"xchain{i}", [128, KT, S], F32, kind="Internal").ap()
        )
    with tile.TileContext(nc) as tc:
        from contextlib import ExitStack

        with ExitStack() as ctx:
            perm = ctx.enter_context(tc.tile_pool(name="perm", bufs=1))
            consts = {}
            ones_row = perm.tile([1, 512], BF, tag="ones_row")
            nc.vector.memset(ones_row, 1.0)
            ones_col = perm.tile([128, 1], BF, tag="ones_col")
            nc.vector.memset(ones_col, 1.0)
            ones64 = perm.tile([65, 64], BF, tag="ones64")
            nc.vector.memset(ones64[64:65, :], 1.0)
            consts.update(ones_row=ones_row, ones_col=ones_col, ones64=ones64)
            bq_sb = perm.tile([128, KT], F32, tag="bq")
            nc.sync.dma_start(bq_sb, t["bq"].rearrange("(j p) -> p j", p=128))
            bk_sb = perm.tile([128, KT], F32, tag="bk")
            nc.sync.dma_start(bk_sb, t["bk"].rearrange("(j p) -> p j", p=128))
            b1_sb = perm.tile([128, FT], F32, tag="b1")
            nc.sync.dma_start(b1_sb, t["b1x"].rearrange("(j p) -> p j", p=128))
            bd_sb = perm.tile([BN, 1], F32, tag="bd")
            nc.sync.dma_start(bd_sb, t["bdx"].rearrange("(p o) -> p o", o=1))
            bv_row = perm.tile([1, D], BF, tag="bv")
            nc.sync.dma_start(bv_row, t["bvr"])
            bo_row = perm.tile([1, D], BF, tag="bo")
            nc.sync.dma_start(bo_row, t["bor"])
            fb_row = perm.tile([1, D], BF, tag="fb")
            nc.sync.dma_start(fb_row, t["fbr"])
            consts.update(
                bq_sb=bq_sb, bk_sb=bk_sb, b1_sb=b1_sb, bd_sb=bd_sb,
                bv_row=bv_row, bo_row=bo_row, fb_row=fb_row,
            )
            for i in range(dup):
                src = None if i == 0 else chain[i - 1]
                extra = chain[i] if i < dup - 1 else None
                _emit(ctx, tc, t, perm, consts, xt_src=src, out_extra=extra, mode=mode)
    nc.compile()
    return nc


_nc_cache = {}


def _get_nc(dup=1, mode=5):
    key = (dup, mode)
    if key not in _nc_cache:
        _nc_cache[key] = _build(dup, mode)
    return _nc_cache[key]


def _pack_feat(w):
    """[D_in, O] -> [128, D_in//128, O]"""
    din, o = w.shape
    return np.ascontiguousarray(w.reshape(din // 128, 128, o).transpose(1, 0, 2))


def prepare_inputs(inputs):
    """Host-side folding/packing. Returns (shared dict, per-core xt list)."""
    f32 = np.float32
    x = np.asarray(inputs["x"], f32)
    ln1_g, ln1_b = np.asarray(inputs["ln1_g"], f32), np.asarray(inputs["ln1_b"], f32)
    ln2_g, ln2_b = np.asarray(inputs["ln2_g"], f32), np.asarray(inputs["ln2_b"], f32)
    aln_g, aln_b = np.asarray(inputs["aln_g"], f32), np.asarray(inputs["aln_b"], f32)
    wq, wk, wv, wo = (np.asarray(inputs[k], f32) for k in ("wq", "wk", "wv", "wo"))
    w1, w2 = np.asarray(inputs["w1"], f32), np.asarray(inputs["w2"], f32)
    wd, wu = np.asarray(inputs["wd"], f32), np.asarray(inputs["wu"], f32)
    b1, b2 = np.asarray(inputs["b1"], f32), np.asarray(inputs["b2"], f32)
    bd, bu = np.asarray(inputs["bd"], f32), np.asarray(inputs["bu"], f32)
    bo = np.asarray(inputs["bo"], f32)

    shared = {
        "wq": _pack_feat(ln1_g[:, None] * wq).astype(BF_NP),
        "wk": _pack_feat(ln1_g[:, None] * wk).astype(BF_NP),
        "wv": _pack_feat(ln1_g[:, None] * wv).astype(BF_NP),
        "wo": _pack_feat(wo).astype(BF_NP),
        "w1": _pack_feat(ln2_g[:, None] * w1).astype(BF_NP),
        "w2": np.ascontiguousarray(
            w2.reshape(FT, 128, KT, 128).transpose(2, 1, 0, 3)
        ).astype(BF_NP),
        "wd": _pack_feat(aln_g[:, None] * wd).astype(BF_NP),
        "wu": (ASCALE * wu).astype(BF_NP),
        "bq": ln1_b @ wq,
        "bk": ln1_b @ wk,
        "b1x": b1 + ln2_b @ w1,
        "bdx": bd + aln_b @ wd,
        "bvr": (ln1_b @ wv)[None, :].astype(BF_NP),
        "bor": bo[None, :].astype(BF_NP),
        "fbr": (b2 + ASCALE * bu)[None, :].astype(BF_NP),
    }
    xts = [
        np.ascontiguousarray(
            x[c].T.reshape(KT, 128, S).transpose(1, 0, 2)
        )  # [128, KT, S] where [p, kk, s] = x[c, s, kk*128+p]
        for c in range(B)
    ]
    return shared, xts


def unpack_out(packed):
    """[128, KT, S] packed -> [S, D] token-major."""
    return np.ascontiguousarray(
        packed.transpose(1, 0, 2).reshape(D, S).T
    )


def kernel(**inputs):
    nc = _get_nc(dup=1)
    shared, xts = prepare_inputs(inputs)
    in_maps = [{**shared, "xt": xts[c]} for c in range(B)]
    res = run_bass_kernel_spmd(nc, in_maps, core_ids=list(range(B)))
    out = np.stack(
        [unpack_out(res.results[c]["out"]) for c in range(B)], axis=0
    )
    return out.astype(np.float32)


# revision 40
# speedup vs baseline: 6.8376x; 1.7420x over previous
"""Trainium2 Bass kernel for a dense transformer block (B=8,S=1024,D=1024,H=16,FFN=4096)
with a parallel adapter. Data-parallel over batch: one batch element per NeuronCore.

Layout strategy: all activations live "transposed" on chip as [feature_partition, token_free]
([128, n_tiles, S] sbuf tiles). LayerNorm statistics are computed with PE ones-matmuls
(sum and sum-of-squares over the partition/feature axis), broadcast back over partitions
with rank-1 (K=1) matmuls, and applied with two DVE tensor-tensor ops. Attention scores
are computed directly in [key, query] layout so no transposes of the probability matrix
are ever needed; softmax denominators come from a ones-column appended to V; the
probabilities stay unnormalized until after P@V, where a rank-1 broadcast of the
reciprocal row-sum rescales the output. All per-channel biases are injected as rank-1
matmuls into the PSUM accumulation groups.
"""

import sys

sys.path.insert(0, "/opt/trn_rl_repo")

import numpy as np
import ml_dtypes

import concourse.bass as bass  # noqa: F401  (AP types)
import concourse.tile as tile
from concourse import bacc, mybir
from concourse.bass_utils import run_bass_kernel_spmd

BF = mybir.dt.bfloat16
F32 = mybir.dt.float32
BF_NP = ml_dtypes.bfloat16

B, S, D, H, HD, FFN, BN = 8, 1024, 1024, 16, 64, 4096, 64
KT = D // 128  # 8 feature tiles of the model dim
FT = FFN // 128  # 32 feature tiles of the ffn dim
NQ = S // 512  # 512-wide token slices
EPS = 1e-5
SCALE = HD**-0.5
ASCALE = 0.1


def _declare(nc, with_chain_input=False, suffix=""):
    t = {}

    def d(name, shape, dt, kind="ExternalInput"):
        t[name] = nc.dram_tensor(name + suffix, shape, dt, kind=kind).ap()

    d("xt", [128, KT, S], F32)
    d("wq", [128, KT, D], BF)
    d("wk", [128, KT, D], BF)
    d("wv", [128, KT, D], BF)
    d("wo", [128, KT, D], BF)
    d("w1", [128, KT, FFN], BF)
    d("w2", [KT, 128, FT, 128], BF)
    d("wd", [128, KT, BN], BF)
    d("wu", [BN, D], BF)
    d("bq", [D], F32)
    d("bk", [D], F32)
    d("b1x", [FFN], F32)
    d("bdx", [BN], F32)
    d("bvr", [1, D], BF)
    d("bor", [1, D], BF)
    d("fbr", [1, D], BF)
    # packed transposed output [p, kk, s] = out[s, kk*128+p]; host untransposes.
    d("out", [128, KT, S], F32, kind="ExternalOutput")
    return t


def _emit(ctx, tc, t, perm, consts, xt_src=None, out_extra=None, mode=5):
    """Emit one block's worth of instructions.

    t: dict of dram APs. perm: permanent pool. consts: dict with ones tiles + bias tiles.
    xt_src: optional override AP for the xt input (dram, [128, KT, S] f32).
    out_extra: optional dram AP [128, KT, S] f32 that also receives the packed output.
    """
    nc = tc.nc
    ones_row = consts["ones_row"]  # [1, 512] bf16 = 1.0
    ones_col = consts["ones_col"]  # [128, 1] bf16 = 1.0
    ones64 = consts["ones64"]  # [65, 64] bf16, row 64 = 1.0
    bq_sb = consts["bq_sb"]  # [128, KT] f32
    bk_sb = consts["bk_sb"]
    b1_sb = consts["b1_sb"]  # [128, FT] f32
    bd_sb = consts["bd_sb"]  # [BN, 1] f32
    bv_row = consts["bv_row"]  # [1, D] bf16
    bo_row = consts["bo_row"]
    fb_row = consts["fb_row"]

    x2T = perm.tile([128, KT, S], F32, tag="x2T")

    def dump_bf(tile3d, nt):
        """Debug/bisection: stage a [128, nt, S] tile to out (packed layout)."""
        with tc.tile_pool(name="dump", bufs=3) as dp:
            for j in range(min(nt, KT)):
                for qq in range(NQ):
                    ql = slice(qq * 512, (qq + 1) * 512)
                    st = dp.tile([128, 512], F32, tag="st")
                    nc.scalar.copy(st, tile3d[:, j, ql])
                    nc.sync.dma_start(t["out"][:, j, ql], st)
                    if out_extra is not None:
                        nc.sync.dma_start(out_extra[:, j, ql], st)

    def layernorm(src, dst, name):
        """src: [128, KT, S] f32 AP; dst: [128, KT, S] bf16 tile. Plain LN core
        (gamma/beta are folded into the consumers by the host)."""
        with (
            tc.tile_pool(name=f"ln_{name}", bufs=3) as lp,
            tc.tile_pool(name=f"ln_{name}_r", bufs=1) as lr,
            tc.tile_pool(name=f"lnp_{name}", bufs=1, space="PSUM") as pp,
        ):
            s1 = pp.tile([1, S], F32, tag="s1")
            s2 = pp.tile([1, S], F32, tag="s2")
            for qq in range(NQ):
                ql = slice(qq * 512, (qq + 1) * 512)
                for kk in range(KT):
                    xb = lp.tile([128, 512], BF, tag="xb")
                    nc.scalar.copy(xb, src[:, kk, ql])
                    nc.tensor.matmul(
                        s1[0:1, ql], ones_col, xb,
                        start=kk == 0, stop=kk == KT - 1,
                    )
                for kk in range(KT):
                    xq = lp.tile([128, 512], BF, tag="xq")
                    nc.scalar.square(xq, src[:, kk, ql])
                    nc.tensor.matmul(
                        s2[0:1, ql], ones_col, xq,
                        start=kk == 0, stop=kk == KT - 1,
                    )
            m = lr.tile([1, S], F32, tag="m")
            ex2 = lr.tile([1, S], F32, tag="ex2")
            nc.vector.tensor_scalar_mul(m, s1[0:1, :], 1.0 / D)
            nc.vector.tensor_scalar_mul(ex2, s2[0:1, :], 1.0 / D)
            var = lr.tile([1, S], F32, tag="var")
            nc.vector.tensor_mul(var, m, m)
            nc.vector.tensor_sub(var, ex2, var)
            nc.vector.tensor_scalar_add(var, var, EPS)
            rv = lr.tile([1, S], F32, tag="rv")
            nc.vector.reciprocal(rv, var)
            rstd = lr.tile([1, S], F32, tag="rstd")
            nc.scalar.sqrt(rstd, rv)  # 1/sqrt(var+eps)
            nmrs = lr.tile([1, S], F32, tag="nmrs")
            nc.vector.tensor_mul(nmrs, m, rstd)
            rstd_bf = lr.tile([1, S], BF, tag="rstd_bf")
            nmrs_bf = lr.tile([1, S], BF, tag="nmrs_bf")
            nc.scalar.copy(rstd_bf, rstd)
            nc.scalar.activation(
                nmrs_bf, nmrs, mybir.ActivationFunctionType.Copy, scale=-1.0
            )
            rb = pp.tile([128, S], F32, tag="rb")
            mb = pp.tile([128, S], F32, tag="mb")
            for qq in range(NQ):
                ql = slice(qq * 512, (qq + 1) * 512)
                nc.tensor.matmul(
                    rb[:, ql], ones_row[0:1, 0:128], rstd_bf[0:1, ql],
                    start=True, stop=True,
                )
                nc.tensor.matmul(
                    mb[:, ql], ones_row[0:1, 0:128], nmrs_bf[0:1, ql],
                    start=True, stop=True,
                )
            for kk in range(KT):
                tmp = lp.tile([128, S], F32, tag="lntmp")
                nc.vector.tensor_mul(tmp, src[:, kk, :], rb)
                nc.vector.tensor_add(dst[:, kk, :], tmp, mb)

    # ================= attention =================
    with tc.tile_pool(name="attn_big", bufs=1) as ap_:
        xt_sb = ap_.tile([128, KT, S], F32, tag="xt")
        xsrc = xt_src if xt_src is not None else t["xt"]
        for kk in range(KT):
            eng = nc.sync if kk % 2 == 0 else nc.gpsimd
            eng.dma_start(xt_sb[:, kk, :], xsrc[:, kk, :])
        if mode == 0:
            dump_bf(xt_sb, KT)
            return
        qT = ap_.tile([128, KT, S], BF, tag="qT")
        kT = ap_.tile([128, KT, S], BF, tag="kT")
        vS = ap_.tile([128, KT, H, HD + 1], BF, tag="vS")  # token-major V + ones col
        attnT = ap_.tile([128, KT, S], BF, tag="attnT")
        nc.vector.memset(vS[:, :, :, HD : HD + 1], 1.0)

        with tc.tile_pool(name="hT", bufs=1) as hp:
            hT = hp.tile([128, KT, S], BF, tag="hT")
            layernorm(xt_sb, hT, "ln1")
            with (
                tc.tile_pool(name="wqkv", bufs=1) as wp,
                tc.tile_pool(name="psqkv", bufs=4, space="PSUM") as qp,
            ):
                wq_sb = wp.tile([128, KT, D], BF, tag="wq")
                wk_sb = wp.tile([128, KT, D], BF, tag="wk")
                wv_sb = wp.tile([128, KT, D], BF, tag="wv")
                for kk in range(KT):  # chunked across both HWDGE engines
                    eng = nc.sync if kk % 2 == 0 else nc.gpsimd
                    eng.dma_start(wq_sb[:, kk, :], t["wq"][:, kk, :])
                    eng.dma_start(wk_sb[:, kk, :], t["wk"][:, kk, :])
                    eng.dma_start(wv_sb[:, kk, :], t["wv"][:, kk, :])
                for j in range(KT):
                    jl = slice(j * 128, (j + 1) * 128)
                    psq = qp.tile([128, S], F32, tag="ps")
                    psk = qp.tile([128, S], F32, tag="ps")
                    for kk in range(KT):
                        for ss in range(NQ):  # weight reuse across slices
                            sl = slice(ss * 512, (ss + 1) * 512)
                            nc.tensor.matmul(
                                psq[:, sl], wq_sb[:, kk, jl], hT[:, kk, sl],
                                start=kk == 0, stop=kk == KT - 1,
                            )
                        for ss in range(NQ):
                            sl = slice(ss * 512, (ss + 1) * 512)
                            nc.tensor.matmul(
                                psk[:, sl], wk_sb[:, kk, jl], hT[:, kk, sl],
                                start=kk == 0, stop=kk == KT - 1,
                            )
                    nc.vector.tensor_scalar_add(qT[:, j, :], psq, bq_sb[:, j : j + 1])
                    nc.vector.tensor_scalar_add(kT[:, j, :], psk, bk_sb[:, j : j + 1])
                for si in range(KT):
                    il = slice(si * 128, (si + 1) * 128)
                    psv = qp.tile([128, S], F32, tag="ps")
                    for kk in range(KT):
                        for dd in range(NQ):  # hT stationary reused
                            dl = slice(dd * 512, (dd + 1) * 512)
                            nc.tensor.matmul(
                                psv[:, dl], hT[:, kk, il], wv_sb[:, kk, dl],
                                start=kk == 0, stop=False,
                            )
                    for dd in range(NQ):
                        dl = slice(dd * 512, (dd + 1) * 512)
                        nc.tensor.matmul(
                            psv[:, dl], ones_row[0:1, 0:128], bv_row[0:1, dl],
                            start=False, stop=True,
                        )
                    nc.vector.tensor_copy(
                        vS[:, si, :, 0:HD],
                        psv.rearrange("p (h e) -> p h e", h=H),
                    )

        if mode == 1:
            dump_bf(qT, KT)
            return

        # scores -> exp -> P@V -> normalize
        with (
            tc.tile_pool(name="pt", bufs=1) as ptp,
            tc.tile_pool(name="att_sm", bufs=2) as smp,
            tc.tile_pool(name="rs_dram", bufs=2, space="DRAM") as rdp,
            tc.tile_pool(name="ps_sc", bufs=2, space="PSUM") as scp,
            tc.tile_pool(name="ps_pv", bufs=2, space="PSUM") as pvp,
        ):
            for tp in range(KT):  # head pair
                ptA = ptp.tile([128, KT, S], BF, tag="ptA")
                ptB = ptp.tile([128, KT, S], BF, tag="ptB")
                for kk in range(KT):
                    kl = slice(kk * 128, (kk + 1) * 128)
                    psA = scp.tile([128, S], F32, tag="sc")
                    psB = scp.tile([128, S], F32, tag="sc")
                    for qq in range(NQ):  # kT stationary reused; A/B row-packed
                        ql = slice(qq * 512, (qq + 1) * 512)
                        nc.tensor.matmul(
                            psA[:, ql], kT[0:64, tp, kl], qT[0:64, tp, ql],
                            start=True, stop=True,
                        )
                    for qq in range(NQ):
                        ql = slice(qq * 512, (qq + 1) * 512)
                        nc.tensor.matmul(
                            psB[:, ql], kT[64:128, tp, kl], qT[64:128, tp, ql],
                            start=True, stop=True,
                        )
                    nc.scalar.activation(
                        ptA[:, kk, :], psA,
                        mybir.ActivationFunctionType.Exp, scale=SCALE,
                    )
                    nc.scalar.activation(
                        ptB[:, kk, :], psB,
                        mybir.ActivationFunctionType.Exp, scale=SCALE,
                    )
                pvA = pvp.tile([65, S], F32, tag="pv")
                pvB = pvp.tile([65, S], F32, tag="pv")
                for kk in range(KT):
                    for qq in range(NQ):  # V stationary reused
                        ql = slice(qq * 512, (qq + 1) * 512)
                        nc.tensor.matmul(
                            pvA[:, ql], vS[:, kk, 2 * tp, :], ptA[:, kk, ql],
                            start=kk == 0, stop=kk == KT - 1,
                        )
                    for qq in range(NQ):
                        ql = slice(qq * 512, (qq + 1) * 512)
                        nc.tensor.matmul(
                            pvB[:, ql], vS[:, kk, 2 * tp + 1, :], ptB[:, kk, ql],
                            start=kk == 0, stop=kk == KT - 1,
                        )
                for h01, pv in ((0, pvA), (1, pvB)):
                    au = smp.tile([64, S], BF, tag="au")
                    nc.vector.tensor_copy(au, pv[0:64, :])
                    rr = smp.tile([65, S], F32, tag="rr")
                    nc.vector.reciprocal(rr[64:65, :], pv[64:65, :])
                    rb = smp.tile([64, S], F32, tag="rbc")
                    # broadcast the reciprocal row over 64 partitions via a
                    # DRAM round-trip (DRAM APs support step-0 partition dims)
                    rs_d = rdp.tile([1, S], F32, tag="rsd")
                    nc.gpsimd.dma_start(rs_d, rr[64:65, :])
                    rs_b = bass.AP(
                        tensor=rs_d.tensor, offset=rs_d.offset,
                        ap=[[0, 64]] + list(rs_d.ap[1:]),
                    )
                    nc.gpsimd.dma_start(rb, rs_b)
                    if h01 == 0:
                        nc.vector.tensor_mul(attnT[0:64, tp, :], au, rb)
                    else:
                        tmp2 = smp.tile([64, S], BF, tag="tmp2")
                        nc.vector.tensor_mul(tmp2, au, rb)
                        nc.gpsimd.dma_start(attnT[64:128, tp, :], tmp2)

        if mode == 2:
            dump_bf(attnT, KT)
            return

        # out-projection + residual -> x2T (f32)
        with (
            tc.tile_pool(name="wo", bufs=1) as wop,
            tc.tile_pool(name="ps_wo", bufs=4, space="PSUM") as wpp,
        ):
            wo_sb = wop.tile([128, KT, D], BF, tag="wo")
            for kk in range(KT):
                eng = nc.sync if kk % 2 == 0 else nc.gpsimd
                eng.dma_start(wo_sb[:, kk, :], t["wo"][:, kk, :])
            for j in range(KT):
                jl = slice(j * 128, (j + 1) * 128)
                ps = wpp.tile([128, S], F32, tag="ps")
                for kk in range(KT):
                    for qq in range(NQ):
                        ql = slice(qq * 512, (qq + 1) * 512)
                        nc.tensor.matmul(
                            ps[:, ql], wo_sb[:, kk, jl], attnT[:, kk, ql],
                            start=kk == 0, stop=False,
                        )
                for qq in range(NQ):
                    ql = slice(qq * 512, (qq + 1) * 512)
                    nc.tensor.matmul(
                        ps[:, ql], bo_row[0:1, jl], ones_row[0:1, 0:512],
                        start=False, stop=True,
                    )
                nc.vector.tensor_add(x2T[:, j, :], xt_sb[:, j, :], ps)

    if mode == 3:
        dump_bf(x2T, KT)
        return

    # ================= MLP + adapter =================
    with tc.tile_pool(name="ffn_big", bufs=1) as fbp:
        zT = fbp.tile([128, FT, S], BF, tag="zT")
        rT = fbp.tile([BN, S], BF, tag="rT")
        with tc.tile_pool(name="nT", bufs=1) as ntp:
            nT = ntp.tile([128, KT, S], BF, tag="nT")
            layernorm(x2T, nT, "ln2")
            with (
                tc.tile_pool(name="w1s", bufs=2) as w1p,
                tc.tile_pool(name="ps_u", bufs=3, space="PSUM") as pup,
            ):
                for fq in range(4):  # stream w1 in quarters
                    w1_q = w1p.tile([128, KT, 1024], BF, tag="w1q")
                    for kk in range(KT):
                        eng = nc.sync if kk % 2 == 0 else nc.gpsimd
                        eng.dma_start(
                            w1_q[:, kk, :],
                            t["w1"][:, kk, fq * 1024 : (fq + 1) * 1024],
                        )
                    for fl in range(8):
                        f = fq * 8 + fl
                        fsl = slice(fl * 128, (fl + 1) * 128)
                        psu = pup.tile([128, S], F32, tag="pu")
                        for kk in range(KT):
                            for ss in range(NQ):  # w1 stationary reused
                                sl = slice(ss * 512, (ss + 1) * 512)
                                nc.tensor.matmul(
                                    psu[:, sl], w1_q[:, kk, fsl], nT[:, kk, sl],
                                    start=kk == 0, stop=kk == KT - 1,
                                )
                        nc.scalar.activation(
                            zT[:, f, :], psu,
                            mybir.ActivationFunctionType.Gelu,
                            bias=b1_sb[:, f : f + 1],
                        )
                # adapter down + relu
                with tc.tile_pool(name="wds", bufs=1) as wdp:
                    wd_sb = wdp.tile([128, KT, BN], BF, tag="wd")
                    nc.sync.dma_start(wd_sb, t["wd"])
                    psd = pup.tile([BN, S], F32, tag="pd", bufs=1)
                    for kk in range(KT):
                        for qq in range(NQ):
                            ql = slice(qq * 512, (qq + 1) * 512)
                            nc.tensor.matmul(
                                psd[:, ql], wd_sb[:, kk, :], nT[:, kk, ql],
                                start=kk == 0, stop=kk == KT - 1,
                            )
                    nc.scalar.activation(
                        rT[0:BN, :], psd,
                        mybir.ActivationFunctionType.Relu,
                        bias=bd_sb[:, 0:1],
                    )
        if mode == 4:
            dump_bf(zT, KT)
            return

        with (
            tc.tile_pool(name="w2s", bufs=2) as w2p,
            tc.tile_pool(name="wus", bufs=1) as wup,
            tc.tile_pool(name="outs", bufs=3) as otp,
            tc.tile_pool(name="ps_y", bufs=4, space="PSUM") as pyp,
        ):
            wu_sb = wup.tile([BN, D], BF, tag="wu")
            nc.sync.dma_start(wu_sb, t["wu"])
            for j in range(KT):
                jl = slice(j * 128, (j + 1) * 128)
                w2_j = w2p.tile([128, FT, 128], BF, tag="w2j")
                for fh in range(4):
                    eng = nc.sync if fh % 2 == 0 else nc.gpsimd
                    fsl = slice(fh * 8, (fh + 1) * 8)
                    eng.dma_start(w2_j[:, fsl, :], t["w2"][j][:, fsl, :])
                psy = pyp.tile([128, S], F32, tag="py")
                for f in range(FT):
                    for qq in range(NQ):  # w2 stationary reused
                        ql = slice(qq * 512, (qq + 1) * 512)
                        nc.tensor.matmul(
                            psy[:, ql], w2_j[:, f, :], zT[:, f, ql],
                            start=f == 0, stop=False,
                        )
                for qq in range(NQ):
                    ql = slice(qq * 512, (qq + 1) * 512)
                    nc.tensor.matmul(
                        psy[:, ql], wu_sb[0:BN, jl], rT[0:BN, ql],
                        start=False, stop=False,
                    )
                    nc.tensor.matmul(
                        psy[:, ql], fb_row[0:1, jl], ones_row[0:1, 0:512],
                        start=False, stop=True,
                    )
                ot = otp.tile([128, S], F32, tag="ot")
                nc.vector.tensor_add(ot, x2T[:, j, :], psy)
                eng = nc.sync if j % 2 == 0 else nc.gpsimd
                eng.dma_start(t["out"][:, j, :], ot)
                if out_extra is not None:
                    eng.dma_start(out_extra[:, j, :], ot)


def _build(dup=1, mode=5):
    nc = bacc.Bacc("TRN2", target_bir_lowering=False, debug=False, num_devices=8)
    t = _declare(nc)
    chain = []
    for i in range(max(0, dup - 1)):
        chain.append(
            nc.dram_tensor(f"xchain{i}", [128, KT, S], F32, kind="Internal").ap()
        )
    with tile.TileContext(nc) as tc:
        from contextlib import ExitStack

        with ExitStack() as ctx:
            perm = ctx.enter_context(tc.tile_pool(name="perm", bufs=1))
            consts = {}
            ones_row = perm.tile([1, 512], BF, tag="ones_row")
            nc.vector.memset(ones_row, 1.0)
            ones_col = perm.tile([128, 1], BF, tag="ones_col")
            nc.vector.memset(ones_col, 1.0)
            ones64 = perm.tile([65, 64], BF, tag="ones64")
            nc.vector.memset(ones64[64:65, :], 1.0)
            consts.update(ones_row=ones_row, ones_col=ones_col, ones64=ones64)
            bq_sb = perm.tile([128, KT], F32, tag="bq")
            nc.sync.dma_start(bq_sb, t["bq"].rearrange("(j p) -> p j", p=128))
            bk_sb = perm.tile([128, KT], F32, tag="bk")
            nc.sync.dma_start(bk_sb, t["bk"].rearrange("(j p) -> p j", p=128))
            b1_sb = perm.tile([128, FT], F32, tag="b1")
            nc.sync.dma_start(b1_sb, t["b1x"].rearrange("(j p) -> p j", p=128))
            bd_sb = perm.tile([BN, 1], F32, tag="bd")
            nc.sync.dma_start(bd_sb, t["bdx"].rearrange("(p o) -> p o", o=1))
            bv_row = perm.tile([1, D], BF, tag="bv")
            nc.sync.dma_start(bv_row, t["bvr"])
            bo_row = perm.tile([1, D], BF, tag="bo")
            nc.sync.dma_start(bo_row, t["bor"])
            fb_row = perm.tile([1, D], BF, tag="fb")
            nc.sync.dma_start(fb_row, t["fbr"])
            consts.update(
                bq_sb=bq_sb, bk_sb=bk_sb, b1_sb=b1_sb, bd_sb=bd_sb,
                bv_row=bv_row, bo_row=bo_row, fb_row=fb_row,
            )
            for i in range(dup):
                src = None if i == 0 else chain[i - 1]
                extra = chain[i] if i < dup - 1 else None
                _emit(ctx, tc, t, perm, consts, xt_src=src, out_extra=extra, mode=mode)
    nc.compile()
    return nc


_nc_cache = {}


def _get_nc(dup=1, mode=5):
    key = (dup, mode)
    if key not in _nc_cache:
        _nc_cache[key] = _build(dup, mode)
    return _nc_cache[key]


def _pack_feat(w):
    """[D_in, O] -> [128, D_in//128, O]"""
    din, o = w.shape
    return np.ascontiguousarray(w.reshape(din // 128, 128, o).transpose(1, 0, 2))


def prepare_inputs(inputs):
    """Host-side folding/packing. Returns (shared dict, per-core xt list)."""
    f32 = np.float32
    x = np.asarray(inputs["x"], f32)
    ln1_g, ln1_b = np.asarray(inputs["ln1_g"], f32), np.asarray(inputs["ln1_b"], f32)
    ln2_g, ln2_b = np.asarray(inputs["ln2_g"], f32), np.asarray(inputs["ln2_b"], f32)
    aln_g, aln_b = np.asarray(inputs["aln_g"], f32), np.asarray(inputs["aln_b"], f32)
    wq, wk, wv, wo = (np.asarray(inputs[k], f32) for k in ("wq", "wk", "wv", "wo"))
    w1, w2 = np.asarray(inputs["w1"], f32), np.asarray(inputs["w2"], f32)
    wd, wu = np.asarray(inputs["wd"], f32), np.asarray(inputs["wu"], f32)
    b1, b2 = np.asarray(inputs["b1"], f32), np.asarray(inputs["b2"], f32)
    bd, bu = np.asarray(inputs["bd"], f32), np.asarray(inputs["bu"], f32)
    bo = np.asarray(inputs["bo"], f32)

    shared = {
        "wq": _pack_feat(ln1_g[:, None] * wq).astype(BF_NP),
        "wk": _pack_feat(ln1_g[:, None] * wk).astype(BF_NP),
        "wv": _pack_feat(ln1_g[:, None] * wv).astype(BF_NP),
        "wo": _pack_feat(wo).astype(BF_NP),
        "w1": _pack_feat(ln2_g[:, None] * w1).astype(BF_NP),
        "w2": np.ascontiguousarray(
            w2.reshape(FT, 128, KT, 128).transpose(2, 1, 0, 3)
        ).astype(BF_NP),
        "wd": _pack_feat(aln_g[:, None] * wd).astype(BF_NP),
        "wu": (ASCALE * wu).astype(BF_NP),
        "bq": ln1_b @ wq,
        "bk": ln1_b @ wk,
        "b1x": b1 + ln2_b @ w1,
        "bdx": bd + aln_b @ wd,
        "bvr": (ln1_b @ wv)[None, :].astype(BF_NP),
        "bor": bo[None, :].astype(BF_NP),
        "fbr": (b2 + ASCALE * bu)[None, :].astype(BF_NP),
    }
    xts = [
        np.ascontiguousarray(
            x[c].T.reshape(KT, 128, S).transpose(1, 0, 2)
        )  # [128, KT, S] where [p, kk, s] = x[c, s, kk*128+p]
        for c in range(B)
    ]
    return shared, xts


def unpack_out(packed):
    """[128, KT, S] packed -> [S, D] token-major."""
    return np.ascontiguousarray(
        packed.transpose(1, 0, 2).reshape(D, S).T
    )


def kernel(**inputs):
    nc = _get_nc(dup=1)
    shared, xts = prepare_inputs(inputs)
    in_maps = [{**shared, "xt": xts[c]} for c in range(B)]
    res = run_bass_kernel_spmd(nc, in_maps, core_ids=list(range(B)))
    out = np.stack(
        [unpack_out(res.results[c]["out"]) for c in range(B)], axis=0
    )
    return out.astype(np.float32)
